# revision 12
# baseline (speedup 1.0000x reference)
"""HetConv via vertical Winograd F(2,3) on the grouped-3x3 part, 8 trn2 cores.

Data-parallel over batch (4 images/core). Per (image, chunk-of-128-channels,
band-of-8-rows):
  - DVE computes the vertical input transform V_p (p=0..3) for the band's 4
    output row-pairs: V0=d0-d2, V1=d1+d2, V2=d2-d1, V3=d1-d3 (d = 4 padded
    input rows of the pair).
  - PE accumulates 4 point-PSUMs m_p[128 oc, 4 pairs x 64 cols]:
      m_p += sum_kx U_p[kx] @ V_p[pairs, kx:kx+64]   (3 matmuls, block-diag
      per-group transformed Wk), plus the 1x1 W1 part fused in at zero
      vector cost: +W1 (even out rows) accumulates into m0 and -W1 (odd
      rows) into m3, because y0 = m0+m1+m2 and y1 = m1-m2-m3.
  - Pool does the inverse transform straight into the bf16 output tile:
      t = m1+m2; y_even = m0'+t; s = m2+m3'; y_odd = m1-s.
16 N=256 matmuls per (band, chunk) = 262144 PE row-cycles/core vs 327680
for direct conv. Warm-up matmuls keep PE busy from t~0 for the p-state ramp.
"""
import sys

sys.path.insert(0, "/opt/trn_rl_repo")

import numpy as np
import ml_dtypes
import concourse.bacc as bacc
import concourse.mybir as mybir
from concourse import tile
from concourse.bass_utils import run_bass_kernel_spmd

N_CORES = 8
B, C, H, W = 32, 256, 64, 64
BP = B // N_CORES          # images per core
HP, WP = H + 2, W + 2      # padded image
NBAND = 8                  # bands of 4 output row-pairs (8 rows) per image
NPAIR = 4                  # row-pairs per band
NQ = H // 2                # row-pairs per image (32)
NSLOTS = 32                # 2 chunks x (12 wino + 4 W1) weight slots
NWARM = 40
WARM_N = 128

_PROG = None


def _build(reps=1):
    nc = bacc.Bacc("TRN2", target_bir_lowering=False, debug=False,
                   num_devices=N_CORES)
    f32 = mybir.dt.float32
    bf16 = mybir.dt.bfloat16

    x = nc.dram_tensor("x", [BP, C, HP * WP], bf16, kind="ExternalInput").ap()
    w = nc.dram_tensor("w", [128, NSLOTS * 128], bf16, kind="ExternalInput").ap()
    out = nc.dram_tensor("out", [BP, C, H, W], bf16, kind="ExternalOutput").ap()

    x_r = x.rearrange("b (k four) s -> b four k s", four=4)
    out_r = out.rearrange("b (k four) h w -> b four k h w", four=4)

    with tile.TileContext(nc) as tc:
        with (
            tc.tile_pool(name="wpool", bufs=1) as wpool,
            tc.tile_pool(name="xpool", bufs=4) as xpool,
            tc.tile_pool(name="vpool", bufs=2) as vpool,
            tc.tile_pool(name="opool", bufs=6) as opool,
            tc.tile_pool(name="scrpool", bufs=4) as scrpool,
            tc.tile_pool(name="wmsb", bufs=1) as wmsb,
            tc.tile_pool(name="pspool", bufs=2, space="PSUM") as pspool,
        ):
            wsrc = wmsb.tile([1, WARM_N], bf16)
            nc.gpsimd.memset(wsrc[:, :], 0.0)

            # weight halves: chunk 0's 16 slots first so band-0 chunk-0 can
            # start after ~3us of DMA.
            wt = wpool.tile([128, NSLOTS * 128], bf16)
            nc.sync.dma_start(out=wt[:, :16 * 128], in_=w[:, :16 * 128])

            # image 0 in per-band row pieces (chunk0 on ACT HWDGE, chunk1 on
            # SP HWDGE — NOT on DVE, whose SEQ must stay free for the input
            # transform; a DMACopy holds its SEQ through the shared HWDGE
            # stage). Weights' second half follows the first piece pair.
            xp0 = [xpool.tile([128, HP * WP], bf16, tag=f"xp{c}",
                              name=f"xp0_{c}") for c in (0, 1)]
            piece_rows = [(0, 10)] + [(8 * k + 2, 8 * k + 10)
                                      for k in range(1, NBAND)]
            for pi, (r0, r1) in enumerate(piece_rows):
                for cchunk, eng in ((0, nc.sync), (1, nc.sync)):
                    eng.dma_start(
                        out=xp0[cchunk][:, r0 * WP:r1 * WP],
                        in_=x_r[0, 2 * cchunk:2 * cchunk + 2, :, r0 * WP:r1 * WP],
                    )
                if pi == 0:
                    nc.sync.dma_start(out=wt[:, 16 * 128:],
                                      in_=w[:, 16 * 128:])

            # warm-ups write into one of the ps0 ring buffers (all 8 PSUM
            # banks are taken by the double-buffered point-PSUMs); the WAW
            # dep on the loop's first ps0 reuse is long satisfied by then.
            wps = pspool.tile([128, 4 * NPAIR * 64], f32, tag="ps0",
                              name="wps")
            for _ in range(NWARM):
                nc.tensor.matmul(wps[0:1, 0:WARM_N], wsrc[0:1, 0:1],
                                 wsrc[0:1, :], start=True, stop=True)

            def wslot(c, s):
                # chunk c slot s: 0-11 wino (p*3+kx), 12/13 W1even ic-chunk
                # 0/1, 14/15 W1odd(-) ic-chunk 0/1
                i = 16 * c + s
                return wt[:, i * 128:(i + 1) * 128]

            def vops(xt, vt, q0, q1):
                xv = xt[:, :].rearrange("p (h w) -> p h w", w=WP)
                vv = vt[:, :].rearrange("p (pt q w) -> p pt q w", q=NQ, w=WP)
                d = [xv[:, 2 * q0 + k:2 * (q1 - 1) + k + 1:2, :]
                     for k in range(4)]
                nc.vector.tensor_sub(vv[:, 0, q0:q1], d[0], d[2])
                nc.vector.tensor_add(vv[:, 1, q0:q1], d[1], d[2])
                nc.vector.tensor_sub(vv[:, 2, q0:q1], d[2], d[1])
                nc.vector.tensor_sub(vv[:, 3, q0:q1], d[1], d[3])

            # image 0's V per band, right behind the piece DMAs
            xps = xp0
            vts = [vpool.tile([128, 4 * NQ * WP], bf16, tag=f"v{c}",
                              name=f"v0_{c}") for c in (0, 1)]
            for nt in range(NBAND):
                for c in (0, 1):
                    vops(xps[c], vts[c], NPAIR * nt, NPAIR * (nt + 1))

            def emit_inverse(img, nt, cb, ot):
                cbv = cb[:, :].rearrange("p (c pt r w) -> p c pt r w",
                                         c=2, pt=4, w=W)
                otv = ot[:, :].rearrange("p (c r w) -> p c r w", c=2, w=W)
                ye = otv[:, :, 0:8:2, :]
                yo = otv[:, :, 1:8:2, :]
                tsc = scrpool.tile([128, 2 * NPAIR * 64], bf16, tag="ts",
                                   name="tsc")
                vsc = scrpool.tile([128, 2 * NPAIR * 64], bf16, tag="vs",
                                   name="vsc")
                ts3 = tsc[:, :].rearrange("p (c r w) -> p c r w", c=2, w=W)
                vs3 = vsc[:, :].rearrange("p (c r w) -> p c r w", c=2, w=W)
                nc.vector.tensor_add(ts3, cbv[:, :, 1], cbv[:, :, 2])
                nc.vector.tensor_add(ye, cbv[:, :, 0], ts3)
                nc.gpsimd.tensor_sub(vs3, cbv[:, :, 1], cbv[:, :, 2])
                nc.gpsimd.tensor_sub(yo, vs3, cbv[:, :, 3])
                for cchunk in (0, 1):
                    eng = nc.sync if cchunk == 0 else nc.scalar
                    eng.dma_start(
                        out=out_r[img, 2 * cchunk:2 * cchunk + 2, :,
                                  8 * nt:8 * (nt + 1), :],
                        in_=ot[:, 512 * cchunk:512 * (cchunk + 1)],
                    )

            def load_img(i):
                xt2 = [xpool.tile([128, HP * WP], bf16, tag=f"xp{c}",
                                  name=f"xq_{c}") for c in (0, 1)]
                for c in (0, 1):
                    nc.sync.dma_start(out=xt2[c][:, :],
                                      in_=x_r[i, 2 * c:2 * c + 2])
                return xt2

            # with a 4-deep x ring, every image loads with NO buffer-reuse
            # dependency — issue them all up front so the V-prefetch never
            # waits on a load stuck behind SP's ot-waits (that lateness was
            # the ~6.5us PE stall at each image rollover)
            xq = {i: load_img(i) for i in range(1, BP)}

            pending = None
            for img in range(BP):
                xvs = [t[:, :].rearrange("p (h w) -> p h w", w=WP)
                       for t in xps]
                vvs = [t[:, :].rearrange("p (pt q w) -> p pt q w",
                                         q=NQ, w=WP) for t in vts]
                xps_nxt = vts_nxt = None

                for nt in range(NBAND):
                    q0 = NPAIR * nt
                    # band-batched inverse state: one cb/ot per band spanning
                    # both chunks, so the elementwise ops (and their sems)
                    # come at half the count.
                    cb = scrpool.tile([128, 2 * 4 * NPAIR * 64], bf16,
                                      tag="cb", name="cb")
                    ot = opool.tile([128, 2 * 8 * W], bf16, tag="ot",
                                    name="ot")
                    for cchunk in (0, 1):
                        ps = pspool.tile([128, 4 * NPAIR * 64], f32,
                                         tag=f"ps{cchunk}",
                                         name=f"ps_{cchunk}")
                        mp = ps[:, :].rearrange("p (pt n) -> p pt n", pt=4)
                        # taps for m1, m2 first, then m0 (+W1 even), m3
                        # (+W1 odd): the cross-chunk W1 matmuls come last so
                        # image-0 piece DMAs of the other chunk have landed.
                        for p in (1, 2):
                            for kx in range(3):
                                nc.tensor.matmul(
                                    mp[:, p, :], wslot(cchunk, 3 * p + kx),
                                    vvs[cchunk][:, p, q0:q0 + NPAIR, kx:kx + W],
                                    start=(kx == 0), stop=(kx == 2),
                                )
                        for p, wbase, rk in ((0, 12, 1), (3, 14, 2)):
                            for kx in range(3):
                                nc.tensor.matmul(
                                    mp[:, p, :], wslot(cchunk, 3 * p + kx),
                                    vvs[cchunk][:, p, q0:q0 + NPAIR, kx:kx + W],
                                    start=(kx == 0), stop=False,
                                )
                            # W1 part: center-tap rows of parity rk, both
                            # ic chunks; odd slots are host-negated
                            for icc in (0, 1):
                                nc.tensor.matmul(
                                    mp[:, p, :], wslot(cchunk, wbase + icc),
                                    xvs[icc][:, 8 * nt + rk:8 * nt + rk + 8:2,
                                             1:1 + W],
                                    start=False, stop=(icc == 1),
                                )
                        # ACT (the only engine allowed to read PSUM here —
                        # the BIR verifier caps elementwise ops at ONE PSUM
                        # operand and Pool at none) drains the point-PSUMs
                        # to SBUF bf16 in one wide copy per chunk.
                        nc.scalar.copy(cb[:, 1024 * cchunk:1024 * (cchunk + 1)],
                                       ps[:, :])

                    # pipeline image img+1: x loads on SP at band 0 (so the
                    # DMA queue stays behind the startup transfers), V
                    # transform in 4 half-image chunks mid-image so DVE has
                    # it ready before PE rolls over.
                    if img + 1 < BP:
                        if nt == 0:
                            if img + 1 not in xq:
                                xq[img + 1] = load_img(img + 1)
                            xps_nxt = xq[img + 1]
                            vts_nxt = [vpool.tile([128, 4 * NQ * WP], bf16,
                                                  tag=f"v{c}",
                                                  name=f"vn_{c}")
                                       for c in (0, 1)]
                        if nt in (3, 4, 5, 6):
                            c, half = {3: (0, 0), 4: (0, 1), 5: (1, 0),
                                       6: (1, 1)}[nt]
                            vops(xps_nxt[c], vts_nxt[c],
                                 half * (NQ // 2), (half + 1) * (NQ // 2))

                    # inverse transform for the PREVIOUS band: by the time
                    # the in-order DVE/Pool SEQs reach these, the ACT copies
                    # have long landed, so the ops dispatch wait-free instead
                    # of serially blocking each band (which starved the
                    # V-prefetch and stalled PE ~8us at every image rollover).
                    if pending is not None:
                        emit_inverse(*pending)
                    pending = (img, nt, cb, ot)

                xps, vts = xps_nxt, vts_nxt
            if pending is not None:
                emit_inverse(*pending)

    nc.compile()
    return nc


def _get_prog():
    global _PROG
    if _PROG is None:
        _PROG = _build()
    return _PROG


def _prep_weights(Wk, W1):
    idx = [np.arange(g, 256, 4) for g in range(4)]
    wslabs = np.zeros((NSLOTS, 128, 128), np.float32)
    for c in (0, 1):
        base = 16 * c
        for a in (0, 1):
            g = 2 * c + a
            # [oc, ic, ky] for this group's block, tap column kx
            for kx in range(3):
                gk = Wk[np.ix_(idx[g], idx[g])][:, :, :, kx]  # [oc, ic, 3]
                u = [gk[:, :, 0],
                     (gk[:, :, 0] + gk[:, :, 1] + gk[:, :, 2]) * 0.5,
                     (gk[:, :, 0] - gk[:, :, 1] + gk[:, :, 2]) * 0.5,
                     gk[:, :, 2]]
                for p in range(4):
                    wslabs[base + 3 * p + kx,
                           64 * a:64 * a + 64, 64 * a:64 * a + 64] = u[p].T
        # W1 slots: 12/13 = +W1 from ic chunk 0/1; 14/15 = -W1
        for icc in (0, 1):
            blk = np.zeros((128, 128), np.float32)
            for a in (0, 1):          # ic group within ic chunk icc
                for b in (0, 1):      # oc group within oc chunk c
                    ga, gb = 2 * icc + a, 2 * c + b
                    if ga == gb:
                        continue      # same residue group -> Wk, not W1
                    blk[64 * a:64 * a + 64, 64 * b:64 * b + 64] = \
                        W1[np.ix_(idx[gb], idx[ga])].T
            wslabs[base + 12 + icc] = blk
            wslabs[base + 14 + icc] = -blk
    return np.ascontiguousarray(
        wslabs.transpose(1, 0, 2).reshape(128, NSLOTS * 128)
    ).astype(ml_dtypes.bfloat16)


def _make_in_maps(x, Wk, W1):
    w_host = _prep_weights(np.asarray(Wk, np.float32), np.asarray(W1, np.float32))
    xs = np.asarray(x, np.float32)
    xpad = np.zeros((B, C, HP, WP), np.float32)
    xpad[:, :, 1:H + 1, 1:W + 1] = xs
    xpad = xpad.reshape(B, C, HP * WP).astype(ml_dtypes.bfloat16)
    return [
        {"x": np.ascontiguousarray(xpad[i * BP:(i + 1) * BP]), "w": w_host}
        for i in range(N_CORES)
    ]


def _run(x, Wk, W1, **spmd_kwargs):
    nc = _get_prog()
    in_maps = _make_in_maps(x, Wk, W1)
    res = run_bass_kernel_spmd(nc, in_maps, list(range(N_CORES)), **spmd_kwargs)
    outs = np.concatenate(
        [np.asarray(res.results[i]["out"]) for i in range(N_CORES)], axis=0)
    return outs.astype(np.float32), res


def kernel(x, Wk, W1):
    return _run(x, Wk, W1)[0]
